# revision 28
# baseline (speedup 1.0000x reference)
"""Trainium2 Bass kernel for nn_BGNLLLoss (bivariate-Gaussian NLL loss).

Math (per element t,p):
    mux,muy,lsx,lsy,pc = params[t,p,:];  x,y = targets[t,p,:]
    sx=e^lsx, sy=e^lsy, c=tanh(pc), nr=1-c^2
    a=(x-mux)/sx, b=(y-muy)/sy
    nll = min( (a^2+b^2-2abc)/(2nr) + lsx+lsy + 0.5 ln(nr) + ln(2pi), C )
    loss[p] = sum_t nll[t,p],   C = -ln(1e-20)

cosh/sinh reformulation (kills the -pc term and every 1x custom op):
    (a^2+b^2-2abc)/(2nr) = [a cosh(pc) - b sinh(pc)]^2/2 + b^2/2
    nll = min(gs^2 + bh^2 + lsx + lsy + BI - ln(cosh pc), C)
    with gs = [a cosh - b sinh]/sqrt2, bh = b/sqrt2
    min(V,C) = C - relu(C-V);  loss = T*C - sum_t relu((C-BI) - V)

Engine split (per 512-row block; all unit-stride, no custom/GpSimd ops --
GpSimd elementwise both runs ~5x slower than DVE and locks the shared
SBUF port that every DVE tensor_tensor needs for its second operand):
  ScalarE: isxy=e^{-lsx|lsy}/sqrt2 (one merged ACTIVATE), ep=e^{pc}/2,
           em=e^{-pc}/2, gs^2 via Square, relu((C-BI)-V) from PSUM
  VectorE: 9 stock fp16 tensor_tensor ops at 2x + 1 tensor_scalar
           (-ln(cosh) via fp16 exponent-bits affine)
  TensorE: V = gs^2 + bh^2 + lsx + lsy + nlch via bf16 identity-matmul
           accumulation into PSUM (<=512 free elems per matmul, ISA cap);
           frame sum acc[1,512] += ones^T @ relu-out
Inputs are packed host-side into planar 16-bit DRAM (fp16 [T,5,PC]:
x y mux muy pc; bf16 [T,2,PC]: lsx lsy so they feed bf16 matmuls
directly) -- halves HBM traffic (memory-bound regime) and makes every
engine read unit-stride.  Person dim split across 8 cores, no
collectives.
"""

import math
from contextlib import ExitStack

import numpy as np

import concourse.bass as bass
import concourse.bacc as bacc
import concourse.mybir as mybir
import concourse.tile as tile
from concourse import bass_utils

F32 = mybir.dt.float32
F16 = mybir.dt.float16
BF16 = mybir.dt.bfloat16
I16 = mybir.dt.int16
U16 = mybir.dt.uint16
AF = mybir.ActivationFunctionType
ALU = mybir.AluOpType

T = 4096
P = 4096
N_CORES = 8
PC = P // N_CORES          # persons per core = 512
K = 4                      # 128-row subtiles per block
RB = 128 * K               # rows per block = 512
NB = T // RB               # 8 blocks
NPL = 5                    # fp16 planes: x y mux muy pc
PW = NPL * PC              # 2560
W2 = 2 * PC                # bf16 planes: lsx lsy

LOG2PI = math.log(2.0 * math.pi)
LN2 = math.log(2.0)
CADD = LN2 + LOG2PI
CLAMP = -math.log(1e-20)               # 46.0517...
B_HALF = -0.5 * LN2                    # exp bias: /sqrt2
B_LN2 = -LN2                           # exp bias: /2
# fp16 exponent-bits log: for v = 2^e(1+f) > 0, int16 bits(v) = ((e+15)<<10)+m
# so ln(v) ~= (bits/1024 - 15 + cm)*ln2 with cm = 1.5 - 1/ln2 the zero-mean
# mantissa correction.  nlch = -bits(ch2)*LNK16; the constant part BI moves
# into the relu bias: min(V+BI, C) = C - relu((C-BI) - V).
CMEAN = 1.5 - 1.0 / LN2
LNK16 = LN2 / 1024.0
BI = (15.0 - CMEAN) * LN2 - LN2 + CADD
RBIAS = CLAMP - BI


# --------------------------------------------------------------------------
# Kernel body (per core; SPMD -- same program on all 8 cores)
# --------------------------------------------------------------------------
def _emit(ctx: ExitStack, tc: tile.TileContext, pln: bass.AP, pln2: bass.AP,
          ident: bass.AP, loss: bass.AP):
    nc = tc.nc

    iop = ctx.enter_context(tc.tile_pool(name="iop", bufs=3))
    actp = ctx.enter_context(tc.tile_pool(name="actp", bufs=2))
    mid = ctx.enter_context(tc.tile_pool(name="mid", bufs=3))
    single = ctx.enter_context(tc.tile_pool(name="single", bufs=1))
    psv = ctx.enter_context(tc.tile_pool(name="psv", bufs=3, space="PSUM"))
    psl = ctx.enter_context(tc.tile_pool(name="psl", bufs=1, space="PSUM"))

    # ACTIVATE bias constants as a tracked tile (no raw memset + barrier in
    # the preamble)
    cst = single.tile([128, 3], F32)
    for i, val in enumerate((B_HALF, B_LN2, RBIAS)):
        nc.vector.memset(cst[:, i:i + 1], val)
        nc.const_aps.aps[(F32, val)] = cst[:, i:i + 1]

    ident_sb = single.tile([128, 128], BF16)
    ones = single.tile([128, 1], BF16)
    nc.vector.memset(ones[:], 1.0)
    acc = psl.tile([1, PC], F32)

    shb = [128, K, PC]
    ctxs: dict[int, dict] = {}

    def stage_load(blk, kss=(slice(0, K),), lxy_first=False):
        r0 = blk * RB
        if blk not in ctxs:
            ctxs[blk] = {"IN": iop.tile([128, K, PW], F16, tag="in",
                                        name="inb"),
                         "LXY": iop.tile([128, K, W2], BF16, tag="lxy",
                                         name="lxyb")}
        IN, LXY = ctxs[blk]["IN"], ctxs[blk]["LXY"]
        for ks in kss:
            ra, rb = r0 + ks.start * 128, r0 + ks.stop * 128
            kk = ks.stop - ks.start
            v = pln[ra:rb, :].rearrange("(k p) w -> p k w", k=kk, p=128)
            v2 = pln2[ra:rb, :].rearrange("(k p) w -> p k w", k=kk, p=128)
            if lxy_first:
                nc.sync.dma_start(LXY[:, ks, :], v2)
                nc.sync.dma_start(IN[:, ks, :], v)
            else:
                nc.sync.dma_start(IN[:, ks, :], v)
                nc.sync.dma_start(LXY[:, ks, :], v2)

    n_ones = [0]
    TOT_ONES = T // 128

    def stage_act(blk, kss):
        c = ctxs[blk]
        v = c["IN"][:].rearrange("p k (c w) -> p k c w", c=NPL)
        if "isxy" not in c:
            c["isxy"] = actp.tile([128, K, 2, PC], F16, tag="isxy",
                                  name="isxy")
            c["ep"] = actp.tile(shb, F16, tag="ep", name="ep")
            c["em"] = actp.tile(shb, F16, tag="em", name="em")
        for ks in kss:
            pcv = v[:, ks, 4, :]
            nc.scalar.activation(
                c["isxy"][:, ks, :, :].rearrange("p k c w -> p k (c w)"),
                c["LXY"][:, ks, :], AF.Exp, scale=-1.0, bias=B_HALF)
            nc.scalar.activation(c["ep"][:, ks, :], pcv, AF.Exp,
                                 scale=1.0, bias=B_LN2)
            nc.scalar.activation(c["em"][:, ks, :], pcv, AF.Exp,
                                 scale=-1.0, bias=B_LN2)

    def stage_main(blk, kss):
        c = ctxs[blk]
        v = c["IN"][:].rearrange("p k (c w) -> p k c w", c=NPL)

        for ks in kss:
            kk = ks.stop - ks.start
            # per-half working tiles (half the SBUF of block-sized tiles,
            # freeing room for the 3-deep input pool)
            shh = [128, kk, PC]
            nxy = mid.tile([128, kk, 2 * PC], F16, tag="nxy", name="nxy")
            ch2 = mid.tile(shh, F16, tag="ch2", name="ch2")
            sh2 = mid.tile(shh, F16, tag="sh2", name="sh2")
            icx = mid.tile(shh, F16, tag="icx", name="icx")
            pp = mid.tile(shh, F16, tag="pp", name="pp")
            qq = mid.tile(shh, F16, tag="qq", name="qq")
            gs = mid.tile(shh, F16, tag="gs", name="gs")
            bb = mid.tile(shh, F16, tag="bb", name="bb")
            g2 = mid.tile(shh, BF16, tag="g2", name="g2")
            b2 = mid.tile(shh, BF16, tag="b2", name="b2")
            nlch = mid.tile(shh, BF16, tag="nlch", name="nlch")
            nr = mid.tile(shh, BF16, tag="nr", name="nr")

            xy = v[:, ks, 0:2, :].rearrange("p k c w -> p k (c w)")
            muxy = v[:, ks, 2:4, :].rearrange("p k c w -> p k (c w)")
            nxv, nyv = nxy[:, :, 0:PC], nxy[:, :, PC:2 * PC]
            epv, emv = c["ep"][:, ks, :], c["em"][:, ks, :]

            nc.vector.tensor_sub(nxy[:], xy, muxy)                  # nx | ny
            nc.vector.tensor_add(ch2[:], epv, emv)                  # cosh
            nc.vector.tensor_sub(sh2[:], epv, emv)                  # sinh
            nc.vector.tensor_mul(icx[:], c["isxy"][:, ks, 0, :], ch2[:])
            nc.vector.tensor_mul(pp[:], nxv, icx[:])
            nc.vector.tensor_mul(bb[:], nyv, c["isxy"][:, ks, 1, :])
            nc.vector.tensor_mul(qq[:], bb[:], sh2[:])
            nc.vector.tensor_sub(gs[:], pp[:], qq[:])
            nc.vector.tensor_mul(b2[:], bb[:], bb[:])
            if blk == NB - 1 and ks.start >= 2:
                # tail: keep the last clamp chain off the ScalarE queue
                nc.vector.tensor_mul(g2[:], gs[:], gs[:])
            else:
                nc.scalar.activation(g2[:], gs[:], AF.Square)
            # nlch = -ln(cosh) - BI via fp16 exponent-bits affine
            nc.vector.tensor_scalar(nlch[:], ch2[:].bitcast(I16),
                                    -LNK16, None, ALU.mult)

            # V = gs^2 + b^2 + lsx + lsy + nlch (identity-matmul accumulate)
            accV = psv.tile([128, 2, PC], F32, tag="accV", name="accV")
            for j in range(kk):
                k = ks.start + j
                o = accV[:, j, :]
                nc.tensor.matmul(o, ident_sb[:], g2[:, j, :],
                                 start=True, stop=False)
                nc.tensor.matmul(o, ident_sb[:], b2[:, j, :],
                                 start=False, stop=False)
                nc.tensor.matmul(o, ident_sb[:], c["LXY"][:, k, 0:PC],
                                 start=False, stop=False)
                nc.tensor.matmul(o, ident_sb[:], c["LXY"][:, k, PC:2 * PC],
                                 start=False, stop=False)
                nc.tensor.matmul(o, ident_sb[:], nlch[:, j, :],
                                 start=False, stop=True)
            # nr = relu((C-BI) - V);  min(V+BI, C) = C - nr
            nc.scalar.activation(nr[:].rearrange("p k n -> p (k n)"),
                                 accV[:, 0:kk, :].rearrange("p k n -> p (k n)"),
                                 AF.Relu, scale=-1.0, bias=RBIAS)
            for j in range(kk):
                nc.tensor.matmul(
                    acc[:, :], ones[:, :], nr[:, j, :],
                    start=(n_ones[0] == 0),
                    stop=(n_ones[0] == TOT_ONES - 1),
                )
                n_ones[0] += 1
        if kss[-1].stop == K:
            del ctxs[blk]

    # Pipelined emission: DMA + ACT front for blk+1 queued ahead of the
    # DVE/PE consumers of blk.  The first block ramps in at half-block
    # granularity; the last block ramps out at 128-row granularity.
    stage_load(0, kss=[slice(0, 1)], lxy_first=True)
    stage_act(0, [slice(0, 1)])
    stage_load(0, kss=[slice(1, 2)], lxy_first=True)
    stage_act(0, [slice(1, 2)])
    nc.sync.dma_start(ident_sb[:], ident)
    stage_load(0, kss=[slice(2, 4)])
    stage_act(0, [slice(2, 4)])
    for i in range(NB):
        if i + 1 < NB:
            stage_load(i + 1)
            stage_act(i + 1, [slice(0, 2), slice(2, 4)])
        if i == 0:
            stage_main(i, [slice(0, 1)])
            stage_main(i, [slice(1, 2)])
            stage_main(i, [slice(2, 4)])
        elif i == NB - 1:
            stage_main(i, [slice(0, 2)])
            stage_main(i, [slice(2, 3)])
            stage_main(i, [slice(3, 4)])
        else:
            stage_main(i, [slice(0, 2), slice(2, 4)])

    out_sb = single.tile([1, PC], F32)
    nc.vector.tensor_scalar(out_sb[:], acc[:, :], -1.0, float(T) * CLAMP,
                            ALU.mult, ALU.add)
    nc.sync.dma_start(loss, out_sb[:])


_CACHED_NC = None


def _build_program() -> bass.Bass:
    global _CACHED_NC
    if _CACHED_NC is not None:
        return _CACHED_NC
    nc = bacc.Bacc("TRN2", target_bir_lowering=False, debug=False,
                   enable_asserts=False)
    pln = nc.dram_tensor("pln", [T, PW], F16, kind="ExternalInput").ap()
    pln2 = nc.dram_tensor("pln2", [T, W2], BF16, kind="ExternalInput").ap()
    ident = nc.dram_tensor("ident", [128, 128], BF16, kind="ExternalInput").ap()
    loss = nc.dram_tensor("loss", [1, PC], F32, kind="ExternalOutput").ap()
    with tile.TileContext(nc) as tc:
        with ExitStack() as ctx:
            _emit(ctx, tc, pln, pln2, ident, loss)
    nc.compile()
    _CACHED_NC = nc
    return nc


def make_in_maps(targets: np.ndarray, params: np.ndarray):
    import ml_dtypes
    bf = ml_dtypes.bfloat16
    t16 = np.asarray(targets).astype(np.float16)   # [T, P, 2]
    pr = np.asarray(params)
    p16 = pr[..., (0, 1, 4)].astype(np.float16)    # mux muy pc
    pb = pr[..., 2:4].astype(bf)                   # lsx lsy
    ident = np.eye(128, dtype=bf)
    in_maps = []
    for i in range(N_CORES):
        sl = slice(i * PC, (i + 1) * PC)
        pl = np.empty((T, NPL, PC), dtype=np.float16)
        pl[:, 0, :] = t16[:, sl, 0]
        pl[:, 1, :] = t16[:, sl, 1]
        pl[:, 2, :] = p16[:, sl, 0]
        pl[:, 3, :] = p16[:, sl, 1]
        pl[:, 4, :] = p16[:, sl, 2]
        p2 = np.empty((T, 2, PC), dtype=bf)
        p2[:, 0, :] = pb[:, sl, 0]
        p2[:, 1, :] = pb[:, sl, 1]
        in_maps.append({"pln": pl.reshape(T, PW), "pln2": p2.reshape(T, W2),
                        "ident": ident})
    return in_maps


def run_spmd(targets: np.ndarray, params: np.ndarray, trace: bool = False):
    nc = _build_program()
    in_maps = make_in_maps(targets, params)
    res = bass_utils.run_bass_kernel_spmd(
        nc, in_maps, core_ids=list(range(N_CORES)), trace=trace,
    )
    loss = np.concatenate(
        [res.results[i]["loss"].reshape(PC) for i in range(N_CORES)]
    ).astype(np.float32)
    return loss, res


def kernel(targets: np.ndarray, params: np.ndarray,
           peopleIDs: np.ndarray | None = None) -> np.ndarray:
    loss, _ = run_spmd(targets, params, trace=False)
    return loss


# revision 29
# speedup vs baseline: 1.1730x; 1.1730x over previous
"""Trainium2 Bass kernel for nn_BGNLLLoss (bivariate-Gaussian NLL loss).

Math (per element t,p):
    mux,muy,lsx,lsy,pc = params[t,p,:];  x,y = targets[t,p,:]
    sx=e^lsx, sy=e^lsy, c=tanh(pc), nr=1-c^2
    a=(x-mux)/sx, b=(y-muy)/sy
    nll = min( (a^2+b^2-2abc)/(2nr) + lsx+lsy + 0.5 ln(nr) + ln(2pi), C )
    loss[p] = sum_t nll[t,p],   C = -ln(1e-20)

cosh/sinh reformulation (kills the -pc term and every 1x custom op):
    (a^2+b^2-2abc)/(2nr) = [a cosh(pc) - b sinh(pc)]^2/2 + b^2/2
    nll = min(gs^2 + bh^2 + lsx + lsy + BI - ln(cosh pc), C)
    with gs = [a cosh - b sinh]/sqrt2, bh = b/sqrt2
    min(V,C) = C - relu(C-V);  loss = T*C - sum_t relu((C-BI) - V)

Engine split (per 512-row block; all unit-stride, no custom/GpSimd ops --
GpSimd elementwise both runs ~5x slower than DVE and locks the shared
SBUF port that every DVE tensor_tensor needs for its second operand):
  ScalarE: isxy=e^{-lsx|lsy}/sqrt2 (one merged ACTIVATE), ep=e^{pc}/2,
           em=e^{-pc}/2, gs^2 via Square, relu((C-BI)-V) from PSUM
  VectorE: 9 stock fp16 tensor_tensor ops at 2x + 1 tensor_scalar
           (-ln(cosh) via fp16 exponent-bits affine)
  TensorE: V = gs^2 + bh^2 + lsx + lsy + nlch via bf16 identity-matmul
           accumulation into PSUM (<=512 free elems per matmul, ISA cap);
           frame sum acc[1,512] += ones^T @ relu-out
Inputs are packed host-side into planar 16-bit DRAM (fp16 [T,5,PC]:
x y mux muy pc; bf16 [T,2,PC]: lsx lsy so they feed bf16 matmuls
directly) -- halves HBM traffic (memory-bound regime) and makes every
engine read unit-stride.  Person dim split across 8 cores, no
collectives.
"""

import math
from contextlib import ExitStack

import numpy as np

import concourse.bass as bass
import concourse.bacc as bacc
import concourse.mybir as mybir
import concourse.tile as tile
from concourse import bass_utils

F32 = mybir.dt.float32
F16 = mybir.dt.float16
BF16 = mybir.dt.bfloat16
I16 = mybir.dt.int16
U16 = mybir.dt.uint16
AF = mybir.ActivationFunctionType
ALU = mybir.AluOpType

T = 4096
P = 4096
N_CORES = 8
PC = P // N_CORES          # persons per core = 512
K = 4                      # 128-row subtiles per block
RB = 128 * K               # rows per block = 512
NB = T // RB               # 8 blocks
NPL = 5                    # fp16 planes: x y mux muy pc
PW = NPL * PC              # 2560
W2 = 2 * PC                # bf16 planes: lsx lsy

LOG2PI = math.log(2.0 * math.pi)
LN2 = math.log(2.0)
CADD = LN2 + LOG2PI
CLAMP = -math.log(1e-20)               # 46.0517...
B_HALF = -0.5 * LN2                    # exp bias: /sqrt2
B_LN2 = -LN2                           # exp bias: /2
# fp16 exponent-bits log: for v = 2^e(1+f) > 0, int16 bits(v) = ((e+15)<<10)+m
# so ln(v) ~= (bits/1024 - 15 + cm)*ln2 with cm = 1.5 - 1/ln2 the zero-mean
# mantissa correction.  nlch = -bits(ch2)*LNK16; the constant part BI moves
# into the relu bias: min(V+BI, C) = C - relu((C-BI) - V).
CMEAN = 1.5 - 1.0 / LN2
LNK16 = LN2 / 1024.0
BI = (15.0 - CMEAN) * LN2 - LN2 + CADD
RBIAS = CLAMP - BI


# --------------------------------------------------------------------------
# Kernel body (per core; SPMD -- same program on all 8 cores)
# --------------------------------------------------------------------------
def _emit(ctx: ExitStack, tc: tile.TileContext, pln: bass.AP, pln2: bass.AP,
          ident: bass.AP, loss: bass.AP):
    nc = tc.nc

    iop = ctx.enter_context(tc.tile_pool(name="iop", bufs=3))
    actp = ctx.enter_context(tc.tile_pool(name="actp", bufs=2))
    mid = ctx.enter_context(tc.tile_pool(name="mid", bufs=3))
    single = ctx.enter_context(tc.tile_pool(name="single", bufs=1))
    psv = ctx.enter_context(tc.tile_pool(name="psv", bufs=3, space="PSUM"))
    psl = ctx.enter_context(tc.tile_pool(name="psl", bufs=1, space="PSUM"))

    # ACTIVATE bias constants as a tracked tile (no raw memset + barrier in
    # the preamble)
    cst = single.tile([128, 3], F32)
    for i, val in enumerate((B_HALF, B_LN2, RBIAS)):
        nc.vector.memset(cst[:, i:i + 1], val)
        nc.const_aps.aps[(F32, val)] = cst[:, i:i + 1]

    ident_sb = single.tile([128, 128], BF16)
    ones = single.tile([128, 1], BF16)
    nc.vector.memset(ones[:], 1.0)
    acc = psl.tile([1, PC], F32)

    shb = [128, K, PC]
    ctxs: dict[int, dict] = {}

    def stage_load(blk, kss=(slice(0, K),)):
        r0 = blk * RB
        if blk not in ctxs:
            ctxs[blk] = {"IN": iop.tile([128, K, PW], F16, tag="in",
                                        name="inb"),
                         "LXY": iop.tile([128, K, W2], BF16, tag="lxy",
                                         name="lxyb")}
        IN, LXY = ctxs[blk]["IN"], ctxs[blk]["LXY"]
        for ks in kss:
            ra, rb = r0 + ks.start * 128, r0 + ks.stop * 128
            kk = ks.stop - ks.start
            v = pln[ra:rb, :].rearrange("(k p) w -> p k w", k=kk, p=128)
            nc.sync.dma_start(IN[:, ks, :], v)
            v2 = pln2[ra:rb, :].rearrange("(k p) w -> p k w", k=kk, p=128)
            nc.sync.dma_start(LXY[:, ks, :], v2)

    n_ones = [0]
    TOT_ONES = T // 128

    def stage_act(blk, kss):
        c = ctxs[blk]
        v = c["IN"][:].rearrange("p k (c w) -> p k c w", c=NPL)
        if "isxy" not in c:
            c["isxy"] = actp.tile([128, K, 2, PC], F16, tag="isxy",
                                  name="isxy")
            c["ep"] = actp.tile(shb, F16, tag="ep", name="ep")
            c["em"] = actp.tile(shb, F16, tag="em", name="em")
        for ks in kss:
            pcv = v[:, ks, 4, :]
            nc.scalar.activation(
                c["isxy"][:, ks, :, :].rearrange("p k c w -> p k (c w)"),
                c["LXY"][:, ks, :], AF.Exp, scale=-1.0, bias=B_HALF)
            nc.scalar.activation(c["ep"][:, ks, :], pcv, AF.Exp,
                                 scale=1.0, bias=B_LN2)
            nc.scalar.activation(c["em"][:, ks, :], pcv, AF.Exp,
                                 scale=-1.0, bias=B_LN2)

    def stage_main(blk, kss):
        c = ctxs[blk]
        v = c["IN"][:].rearrange("p k (c w) -> p k c w", c=NPL)

        for ks in kss:
            kk = ks.stop - ks.start
            # per-half working tiles (half the SBUF of block-sized tiles,
            # freeing room for the 3-deep input pool)
            shh = [128, kk, PC]
            nxy = mid.tile([128, kk, 2 * PC], F16, tag="nxy", name="nxy")
            ch2 = mid.tile(shh, F16, tag="ch2", name="ch2")
            sh2 = mid.tile(shh, F16, tag="sh2", name="sh2")
            icx = mid.tile(shh, F16, tag="icx", name="icx")
            pp = mid.tile(shh, F16, tag="pp", name="pp")
            qq = mid.tile(shh, F16, tag="qq", name="qq")
            gs = mid.tile(shh, F16, tag="gs", name="gs")
            bb = mid.tile(shh, F16, tag="bb", name="bb")
            g2 = mid.tile(shh, BF16, tag="g2", name="g2")
            b2 = mid.tile(shh, BF16, tag="b2", name="b2")
            nlch = mid.tile(shh, BF16, tag="nlch", name="nlch")
            nr = mid.tile(shh, BF16, tag="nr", name="nr")

            xy = v[:, ks, 0:2, :].rearrange("p k c w -> p k (c w)")
            muxy = v[:, ks, 2:4, :].rearrange("p k c w -> p k (c w)")
            nxv, nyv = nxy[:, :, 0:PC], nxy[:, :, PC:2 * PC]
            epv, emv = c["ep"][:, ks, :], c["em"][:, ks, :]

            nc.vector.tensor_sub(nxy[:], xy, muxy)                  # nx | ny
            nc.vector.tensor_add(ch2[:], epv, emv)                  # cosh
            nc.vector.tensor_sub(sh2[:], epv, emv)                  # sinh
            nc.vector.tensor_mul(icx[:], c["isxy"][:, ks, 0, :], ch2[:])
            nc.vector.tensor_mul(pp[:], nxv, icx[:])
            nc.vector.tensor_mul(bb[:], nyv, c["isxy"][:, ks, 1, :])
            nc.vector.tensor_mul(qq[:], bb[:], sh2[:])
            nc.vector.tensor_sub(gs[:], pp[:], qq[:])
            nc.vector.tensor_mul(b2[:], bb[:], bb[:])
            if blk == NB - 1 and ks.start >= 2:
                # tail: keep the last clamp chain off the ScalarE queue
                nc.vector.tensor_mul(g2[:], gs[:], gs[:])
            else:
                nc.scalar.activation(g2[:], gs[:], AF.Square)
            # nlch = -ln(cosh) - BI via fp16 exponent-bits affine
            nc.vector.tensor_scalar(nlch[:], ch2[:].bitcast(I16),
                                    -LNK16, None, ALU.mult)

            # V = gs^2 + b^2 + lsx + lsy + nlch (identity-matmul accumulate)
            accV = psv.tile([128, 2, PC], F32, tag="accV", name="accV")
            for j in range(kk):
                k = ks.start + j
                o = accV[:, j, :]
                nc.tensor.matmul(o, ident_sb[:], g2[:, j, :],
                                 start=True, stop=False)
                nc.tensor.matmul(o, ident_sb[:], b2[:, j, :],
                                 start=False, stop=False)
                nc.tensor.matmul(o, ident_sb[:], c["LXY"][:, k, 0:PC],
                                 start=False, stop=False)
                nc.tensor.matmul(o, ident_sb[:], c["LXY"][:, k, PC:2 * PC],
                                 start=False, stop=False)
                nc.tensor.matmul(o, ident_sb[:], nlch[:, j, :],
                                 start=False, stop=True)
            # nr = relu((C-BI) - V);  min(V+BI, C) = C - nr
            nc.scalar.activation(nr[:].rearrange("p k n -> p (k n)"),
                                 accV[:, 0:kk, :].rearrange("p k n -> p (k n)"),
                                 AF.Relu, scale=-1.0, bias=RBIAS)
            for j in range(kk):
                nc.tensor.matmul(
                    acc[:, :], ones[:, :], nr[:, j, :],
                    start=(n_ones[0] == 0),
                    stop=(n_ones[0] == TOT_ONES - 1),
                )
                n_ones[0] += 1
        if kss[-1].stop == K:
            del ctxs[blk]

    # Pipelined emission: DMA + ACT front for blk+1 queued ahead of the
    # DVE/PE consumers of blk.  The first block ramps in at half-block
    # granularity; the last block ramps out at 128-row granularity.
    stage_load(0, kss=[slice(0, 2)])
    stage_act(0, [slice(0, 2)])
    stage_load(0, kss=[slice(2, 4)])
    nc.sync.dma_start(ident_sb[:], ident)
    stage_act(0, [slice(2, 4)])
    for i in range(NB):
        if i + 1 < NB:
            stage_load(i + 1)
            stage_act(i + 1, [slice(0, 2), slice(2, 4)])
        if i == 0:
            stage_main(i, [slice(0, 2)])
            stage_main(i, [slice(2, 4)])
        elif i == NB - 1:
            stage_main(i, [slice(0, 2)])
            stage_main(i, [slice(2, 3)])
            stage_main(i, [slice(3, 4)])
        else:
            stage_main(i, [slice(0, 2), slice(2, 4)])

    out_sb = single.tile([1, PC], F32)
    nc.vector.tensor_scalar(out_sb[:], acc[:, :], -1.0, float(T) * CLAMP,
                            ALU.mult, ALU.add)
    nc.sync.dma_start(loss, out_sb[:])


_CACHED_NC = None


def _build_program() -> bass.Bass:
    global _CACHED_NC
    if _CACHED_NC is not None:
        return _CACHED_NC
    nc = bacc.Bacc("TRN2", target_bir_lowering=False, debug=False,
                   enable_asserts=False)
    pln = nc.dram_tensor("pln", [T, PW], F16, kind="ExternalInput").ap()
    pln2 = nc.dram_tensor("pln2", [T, W2], BF16, kind="ExternalInput").ap()
    ident = nc.dram_tensor("ident", [128, 128], BF16, kind="ExternalInput").ap()
    loss = nc.dram_tensor("loss", [1, PC], F32, kind="ExternalOutput").ap()
    with tile.TileContext(nc) as tc:
        with ExitStack() as ctx:
            _emit(ctx, tc, pln, pln2, ident, loss)
    nc.compile()
    _CACHED_NC = nc
    return nc


def make_in_maps(targets: np.ndarray, params: np.ndarray):
    import ml_dtypes
    bf = ml_dtypes.bfloat16
    t16 = np.asarray(targets).astype(np.float16)   # [T, P, 2]
    pr = np.asarray(params)
    p16 = pr[..., (0, 1, 4)].astype(np.float16)    # mux muy pc
    pb = pr[..., 2:4].astype(bf)                   # lsx lsy
    ident = np.eye(128, dtype=bf)
    in_maps = []
    for i in range(N_CORES):
        sl = slice(i * PC, (i + 1) * PC)
        pl = np.empty((T, NPL, PC), dtype=np.float16)
        pl[:, 0, :] = t16[:, sl, 0]
        pl[:, 1, :] = t16[:, sl, 1]
        pl[:, 2, :] = p16[:, sl, 0]
        pl[:, 3, :] = p16[:, sl, 1]
        pl[:, 4, :] = p16[:, sl, 2]
        p2 = np.empty((T, 2, PC), dtype=bf)
        p2[:, 0, :] = pb[:, sl, 0]
        p2[:, 1, :] = pb[:, sl, 1]
        in_maps.append({"pln": pl.reshape(T, PW), "pln2": p2.reshape(T, W2),
                        "ident": ident})
    return in_maps


def run_spmd(targets: np.ndarray, params: np.ndarray, trace: bool = False):
    nc = _build_program()
    in_maps = make_in_maps(targets, params)
    res = bass_utils.run_bass_kernel_spmd(
        nc, in_maps, core_ids=list(range(N_CORES)), trace=trace,
    )
    loss = np.concatenate(
        [res.results[i]["loss"].reshape(PC) for i in range(N_CORES)]
    ).astype(np.float32)
    return loss, res


def kernel(targets: np.ndarray, params: np.ndarray,
           peopleIDs: np.ndarray | None = None) -> np.ndarray:
    loss, _ = run_spmd(targets, params, trace=False)
    return loss
